# revision 2
# baseline (speedup 1.0000x reference)
"""HMM forward-algorithm (CgpHmm layer) Trainium2 Bass kernel — For_i version.

Same math as the unrolled baseline, but the 2047-step scan is a hardware
loop of 126 iterations x 16 steps (plus peeled head/tail blocks), which
shrinks the program ~30x: faster per-call jit trace, BIR serialization,
compile-cache hashing, and NEFF load.  The JAX persistent compilation
cache is enabled so warm calls skip XLA/NEFF recompilation entirely.

See kernel.py (baseline) for the math/layout derivation:
  y[s, b] state-major in 4 chunks of [128, 8] bf16; A stationary on the
  PE as 16 bf16 [128,128] tiles; per step 16 LDW+MM accumulate
  y' = A^T-chunks @ y into PSUM; DVE multiplies by the emission slice;
  every 16 steps z = colsum(y), loglik += log z, and 1/z is folded into
  the emission of step t+2 (consumed at the top of the next loop body).
"""

import math
import time

import numpy as np
import ml_dtypes

import jax

try:
    jax.config.update("jax_compilation_cache_dir", "/tmp/hmm_jax_cache")
    jax.config.update("jax_persistent_cache_min_compile_time_secs", 0.3)
except Exception:
    pass

import concourse.bass as bass
import concourse.bacc as bacc
import concourse.mybir as mybir
import concourse.tile as tile
from concourse import bass_utils
from concourse.bass import ds

F32 = mybir.dt.float32
BF16 = mybir.dt.bfloat16

B, S, E = 64, 512, 6
NCORES = 8
BS = B // NCORES            # 8 sequences per core
C = S // 128                # 4 state chunks
T_FULL = 2048
NP = 16                     # normalization period == loop block size

Exp = mybir.ActivationFunctionType.Exp
Ln = mybir.ActivationFunctionType.Ln
AX = mybir.AxisListType.X
MAX = mybir.AluOpType.max
ADD = mybir.AluOpType.add
PE_ENG = mybir.EngineType.PE


def build_program(T=T_FULL, staggered=False, hw_loop=True, hints=True):
    assert T % NP == 0
    n_blocks = T // NP          # 128 blocks of 16 steps
    # block 0: t=1..16 (peeled, static em slices, norm at t=15)
    # blocks 1..n_blocks-2: hardware loop, t=b*16+1..b*16+16, norm at +15
    # block n_blocks-1: t=T-15..T-1 (peeled tail, 15 steps, no norm)
    nc = bacc.Bacc("TRN2", target_bir_lowering=False)

    A_k = nc.dram_tensor("A_kernel", [S, S], F32, kind="ExternalInput")
    B_k = nc.dram_tensor("B_kernel", [S, E], F32, kind="ExternalInput")
    I_k = nc.dram_tensor("I_kernel", [S], F32, kind="ExternalInput")
    OH = nc.dram_tensor("oh", [E, T * BS], BF16, kind="ExternalInput")
    CH = nc.dram_tensor("chain", [1, BS], F32, kind="ExternalInput")
    OUT = nc.dram_tensor("out", [1, BS], F32, kind="ExternalOutput")

    with tile.TileContext(nc) as tc:
        with (
            tc.tile_pool(name="singles", bufs=1) as singles,
            tc.tile_pool(name="work", bufs=2) as work,
            tc.tile_pool(name="small", bufs=2) as small,
            tc.tile_pool(name="ypool", bufs=4) as ypool,
            tc.tile_pool(name="ppre", bufs=2, space="PSUM") as ppre,
            tc.tile_pool(name="gpsum", bufs=4, space="PSUM") as gpsum,
            tc.tile_pool(name="zpool", bufs=1, space="PSUM") as zpool,
            tc.tile_pool(name="bpool", bufs=1, space="PSUM") as bpool,
        ):
            # ---------------- load inputs ----------------
            a_in = []
            A_view = A_k[:].rearrange("(c p) s -> c p s", p=128)
            for k in range(C):
                t_ = work.tile([128, S], F32, tag=f"a_in{k}")
                nc.sync.dma_start(out=t_[:], in_=A_view[k])
                a_in.append(t_)

            bkT = singles.tile([E, S], F32, tag="bkT")
            nc.sync.dma_start(out=bkT[:], in_=B_k[:].rearrange("s e -> e s"))

            i_row = singles.tile([1, S], F32, tag="i_row")
            nc.sync.dma_start(out=i_row[:], in_=I_k[:].rearrange("(a s) -> a s", a=1))

            oh_sb = singles.tile([E, T * BS], BF16, tag="oh")
            nc.sync.dma_start(out=oh_sb[:], in_=OH[:])

            # ---------------- A = softmax rows -> bf16 chunks ----------------
            a_sb = []
            for k in range(C):
                negmax = small.tile([128, 1], F32, tag="negmax")
                nc.vector.tensor_reduce(negmax[:], a_in[k][:], axis=AX, op=MAX,
                                        negate=True)
                expd = work.tile([128, S], F32, tag="expd")
                nc.scalar.activation(expd[:], a_in[k][:], Exp, bias=negmax[:, 0:1])
                ssum = small.tile([128, 1], F32, tag="ssum")
                nc.vector.tensor_reduce(ssum[:], expd[:], axis=AX, op=ADD)
                sinv = small.tile([128, 1], F32, tag="sinv")
                nc.vector.reciprocal(sinv[:], ssum[:])
                ab = singles.tile([128, S], BF16, tag=f"a_sb{k}")
                nc.vector.tensor_scalar_mul(ab[:], expd[:], sinv[:, 0:1])
                a_sb.append(ab)

            # ---------------- BmT6 = 6 * softmax(B_kernel) transposed --------
            expT = singles.tile([E, S], F32, tag="expT")
            nc.scalar.activation(expT[:], bkT[:], Exp)
            ones6 = singles.tile([E, 1], F32, tag="ones6")
            nc.vector.memset(ones6[:], 1.0)
            denT = ppre.tile([1, S], F32, tag="ps")
            nc.tensor.matmul(denT[:], ones6[:], expT[:], start=True, stop=True)
            denrT = singles.tile([1, S], F32, tag="denrT")
            nc.vector.reciprocal(denrT[:], denT[:])
            nc.vector.tensor_scalar_mul(denrT[:], denrT[:], 6.0)
            denr6 = singles.tile([E, S], F32, tag="denr6")
            nc.gpsimd.partition_broadcast(denr6[:], denrT[:], channels=E)
            bmT6 = singles.tile([E, S], BF16, tag="bmT6")
            nc.vector.tensor_mul(bmT6[:], expT[:], denr6[:])

            # ---------------- I = softmax(I_kernel); BmI = BmT6 * I ----------
            iexp = singles.tile([1, S], F32, tag="iexp")
            nc.scalar.activation(iexp[:], i_row[:], Exp)
            isum = small.tile([1, 1], F32, tag="isum")
            nc.vector.tensor_reduce(isum[:], iexp[:], axis=AX, op=ADD)
            iinv = small.tile([1, 1], F32, tag="iinv")
            nc.vector.reciprocal(iinv[:], isum[:])
            inorm = singles.tile([1, S], F32, tag="inorm")
            nc.vector.tensor_scalar_mul(inorm[:], iexp[:], iinv[:, 0:1])
            i6 = singles.tile([E, S], F32, tag="i6")
            nc.gpsimd.partition_broadcast(i6[:], inorm[:], channels=E)
            denr6i = singles.tile([E, S], F32, tag="denr6i")
            nc.vector.tensor_mul(denr6i[:], denr6[:], i6[:])
            bmI = singles.tile([E, S], BF16, tag="bmI")
            nc.vector.tensor_mul(bmI[:], expT[:], denr6i[:])

            # ---------------- emission table (bf16, SBUF-resident) ----------
            em_sb = singles.tile([128, C, T * BS], BF16, tag="em")
            n_tb = (T * BS) // 512
            for m in range(C):
                lhs = bmT6[:, m * 128:(m + 1) * 128]
                for tb in range(n_tb):
                    ps = ppre.tile([128, 512], F32, tag="ps")
                    nc.tensor.matmul(ps[:], lhs, oh_sb[:, tb * 512:(tb + 1) * 512],
                                     start=True, stop=True)
                    dst = em_sb[:, m, tb * 512:(tb + 1) * 512]
                    if tb % 2 == 0:
                        nc.vector.tensor_copy(dst, ps[:])
                    else:
                        nc.scalar.copy(dst, ps[:])

            # ---------------- constants / state ----------------
            ones_col = singles.tile([128, 1], BF16, tag="ones_col")
            nc.vector.memset(ones_col[:], 1.0)
            ones_row = singles.tile([1, 128], F32, tag="ones_row")
            nc.vector.memset(ones_row[:], 1.0)
            loglik = singles.tile([1, BS], F32, tag="loglik")
            nc.vector.memset(loglik[:], 0.0)
            em2 = singles.tile([128, C, BS], BF16, tag="em2")

            # ---------------- y0 = I * em_0 (via BmI one-hot matmul) --------
            y_cur = ypool.tile([128, C, BS], BF16, tag="y")
            for m in range(C):
                ps0 = gpsum.tile([128, BS], F32, tag="g", name="g")
                nc.tensor.matmul(ps0[:], bmI[:, m * 128:(m + 1) * 128],
                                 oh_sb[:, 0:BS], start=True, stop=True)
                nc.vector.tensor_copy(y_cur[:, m, :], ps0[:])

            # one scan step: y_next[m] = (sum_k A^T[m,k] @ y[k]) * em_src(m)
            def step(y, em_src_fn):
                y_next = ypool.tile([128, C, BS], BF16, tag="y")
                for m in range(C):
                    g = gpsum.tile([128, BS], F32, tag="g", name="g")
                    for k in range(C):
                        nc.tensor.matmul(
                            g[:],
                            a_sb[k][:, m * 128:(m + 1) * 128],
                            y[:, k, :],
                            start=(k == 0), stop=(k == C - 1),
                        )
                    nc.vector.tensor_mul(y_next[:, m, :], g[:], em_src_fn(m))
                return y_next

            # z = colsum(y); loglik += ln z; em2 = em[fold_slice] / z
            def norm_and_fold(y, fold_src_fn):
                zp = zpool.tile([1, BS], F32, tag="z")
                for k in range(C):
                    nc.tensor.matmul(zp[:], ones_col[:], y[:, k, :],
                                     start=(k == 0), stop=(k == C - 1))
                zlog = small.tile([1, BS], F32, tag="zlog")
                nc.scalar.activation(zlog[:], zp[:], Ln)
                nc.vector.tensor_add(loglik[:], loglik[:], zlog[:])
                zrec = small.tile([1, BS], F32, tag="zrec")
                nc.vector.reciprocal(zrec[:], zp[:])
                bp = bpool.tile([128, BS], F32, tag="bp")
                nc.tensor.matmul(bp[:], ones_row[:], zrec[:], start=True,
                                 stop=True)
                for m in range(C):
                    nc.vector.tensor_mul(em2[:, m, :], fold_src_fn(m), bp[:])

            # ---------------- peeled head block: t = 1..16 ----------------
            for s in range(1, NP + 1):
                t = s
                y_cur = step(y_cur, lambda m, t=t: em_sb[:, m, t * BS:(t + 1) * BS])
                if s == NP - 1:
                    norm_and_fold(
                        y_cur,
                        lambda m, t=t: em_sb[:, m, (t + 2) * BS:(t + 3) * BS])

            # ---------------- hardware loop: blocks 1..n_blocks-2 ----------
            # loop var 'base' = block_index * NP * BS (element offset into the
            # time-major free axis); covers t = blk*16+1 .. blk*16+16.
            def loop_body(base):
                nonlocal y_cur
                for s in range(1, NP + 1):
                    if s == 1:
                        y_cur = step(y_cur, lambda m: em2[:, m, :])
                    else:
                        y_cur = step(
                            y_cur,
                            lambda m, s=s: em_sb[:, m, ds(base + s * BS, BS)])
                    if s == NP - 1:
                        norm_and_fold(
                            y_cur,
                            lambda m, s=s: em_sb[:, m,
                                                 ds(base + (s + 2) * BS, BS)])

            if hw_loop:
                with tc.For_i(NP * BS, (n_blocks - 1) * NP * BS, NP * BS,
                              hint_engines=(PE_ENG,) if hints else (),
                              staggered_reset=staggered) as base:
                    loop_body(base)
            else:
                for blk in range(1, n_blocks - 1):
                    loop_body(blk * NP * BS)

            # ---------------- peeled tail block: t = T-15..T-1 --------------
            for s in range(1, NP):
                t = (n_blocks - 1) * NP + s
                if s == 1:
                    y_cur = step(y_cur, lambda m: em2[:, m, :])
                else:
                    y_cur = step(y_cur,
                                 lambda m, t=t: em_sb[:, m, t * BS:(t + 1) * BS])

            # ---------------- finalize ----------------
            zf = zpool.tile([1, BS], F32, tag="z")
            for k in range(C):
                nc.tensor.matmul(zf[:], ones_col[:], y_cur[:, k, :],
                                 start=(k == 0), stop=(k == C - 1))
            zflog = small.tile([1, BS], F32, tag="zlog")
            nc.scalar.activation(zflog[:], zf[:], Ln)
            nc.vector.tensor_add(loglik[:], loglik[:], zflog[:])
            nc.vector.tensor_scalar_add(loglik[:], loglik[:],
                                        -float(T) * math.log(6.0))

            # serialization token: loglik += 0 * chain
            ch_sb = singles.tile([1, BS], F32, tag="ch")
            nc.sync.dma_start(out=ch_sb[:], in_=CH[:])
            chz = small.tile([1, BS], F32, tag="chz")
            nc.vector.tensor_scalar_mul(chz[:], ch_sb[:], 0.0)
            nc.vector.tensor_add(loglik[:], loglik[:], chz[:])

            nc.sync.dma_start(out=OUT[:], in_=loglik[:])

    nc.compile()
    return nc


def make_onehot(obs_shard: np.ndarray, T: int) -> np.ndarray:
    """obs_shard [BS, T] ints -> one-hot [E, T*BS] bf16 with oh[e, t*BS+b]."""
    oh = np.zeros((E, T * BS), dtype=ml_dtypes.bfloat16)
    obs = np.asarray(obs_shard).astype(np.int64)
    tb = np.arange(T)[None, :] * BS + np.arange(BS)[:, None]  # [BS, T]
    oh[obs.reshape(-1), tb.reshape(-1)] = 1.0
    return oh


_CACHED = {}


def _get_program(T):
    if T not in _CACHED:
        t0 = time.time()
        _CACHED[T] = build_program(T)
        print(f"[kernel] built bass program T={T} in {time.time()-t0:.1f}s",
              flush=True)
    return _CACHED[T]


def kernel(obs, A_kernel, B_kernel, I_kernel, _trace=False):
    obs = np.asarray(obs)
    A_kernel = np.asarray(A_kernel, dtype=np.float32)
    B_kernel = np.asarray(B_kernel, dtype=np.float32)
    I_kernel = np.asarray(I_kernel, dtype=np.float32)
    Bfull, T = obs.shape
    assert Bfull == B

    nc = _get_program(T)

    in_maps = []
    for c in range(NCORES):
        shard = obs[c * BS:(c + 1) * BS]
        in_maps.append({
            "A_kernel": A_kernel,
            "B_kernel": B_kernel,
            "I_kernel": I_kernel,
            "oh": make_onehot(shard, T),
            "chain": np.zeros((1, BS), np.float32),
        })

    kw = {"trace": True} if _trace else {}
    res = bass_utils.run_bass_kernel_spmd(
        nc, in_maps, core_ids=list(range(NCORES)), **kw,
    )
    out = np.concatenate([r["out"].reshape(BS) for r in res.results])
    kernel._last_result = res
    return out.astype(np.float32)


# revision 3
# speedup vs baseline: 1.1422x; 1.1422x over previous
"""HMM forward-algorithm (CgpHmm layer) Trainium2 Bass kernel — For_i version.

Same math as the unrolled baseline, but the 2047-step scan is a hardware
loop of 126 iterations x 16 steps (plus peeled head/tail blocks), which
shrinks the program ~30x: faster per-call jit trace, BIR serialization,
compile-cache hashing, and NEFF load.  The JAX persistent compilation
cache is enabled so warm calls skip XLA/NEFF recompilation entirely.

See kernel.py (baseline) for the math/layout derivation:
  y[s, b] state-major in 4 chunks of [128, 8] bf16; A stationary on the
  PE as 16 bf16 [128,128] tiles; per step 16 LDW+MM accumulate
  y' = A^T-chunks @ y into PSUM; DVE multiplies by the emission slice;
  every 16 steps z = colsum(y), loglik += log z, and 1/z is folded into
  the emission of step t+2 (consumed at the top of the next loop body).
"""

import math
import time

import numpy as np
import ml_dtypes

import jax

try:
    jax.config.update("jax_compilation_cache_dir", "/tmp/hmm_jax_cache")
    jax.config.update("jax_persistent_cache_min_compile_time_secs", 0.3)
except Exception:
    pass

import concourse.bass as bass
import concourse.bacc as bacc
import concourse.mybir as mybir
import concourse.tile as tile
from concourse import bass_utils
from concourse.bass import ds

F32 = mybir.dt.float32
BF16 = mybir.dt.bfloat16

B, S, E = 64, 512, 6
NCORES = 8
BS = B // NCORES            # 8 sequences per core
C = S // 128                # 4 state chunks
T_FULL = 2048
NP = 16                     # normalization period == loop block size

Exp = mybir.ActivationFunctionType.Exp
Ln = mybir.ActivationFunctionType.Ln
AX = mybir.AxisListType.X
MAX = mybir.AluOpType.max
ADD = mybir.AluOpType.add
PE_ENG = mybir.EngineType.PE


def build_program(T=T_FULL, staggered=False, hw_loop=True, hints=True, fold=True):
    assert T % NP == 0
    n_blocks = T // NP          # 128 blocks of 16 steps
    # block 0: t=1..16 (peeled, static em slices, norm at t=15)
    # blocks 1..n_blocks-2: hardware loop, t=b*16+1..b*16+16, norm at +15
    # block n_blocks-1: t=T-15..T-1 (peeled tail, 15 steps, no norm)
    nc = bacc.Bacc("TRN2", target_bir_lowering=False)

    A_k = nc.dram_tensor("A_kernel", [S, S], F32, kind="ExternalInput")
    B_k = nc.dram_tensor("B_kernel", [S, E], F32, kind="ExternalInput")
    I_k = nc.dram_tensor("I_kernel", [S], F32, kind="ExternalInput")
    OH = nc.dram_tensor("oh", [E, T * BS], BF16, kind="ExternalInput")
    CH = nc.dram_tensor("chain", [1, BS], F32, kind="ExternalInput")
    OUT = nc.dram_tensor("out", [1, BS], F32, kind="ExternalOutput")

    with tile.TileContext(nc) as tc:
        with (
            tc.tile_pool(name="singles", bufs=1) as singles,
            tc.tile_pool(name="work", bufs=2) as work,
            tc.tile_pool(name="small", bufs=2) as small,
            tc.tile_pool(name="ypool", bufs=4) as ypool,
            tc.tile_pool(name="ppre", bufs=2, space="PSUM") as ppre,
            tc.tile_pool(name="gpsum", bufs=4, space="PSUM") as gpsum,
            tc.tile_pool(name="yscale", bufs=1) as yscale,
            tc.tile_pool(name="zpool", bufs=1, space="PSUM") as zpool,
            tc.tile_pool(name="bpool", bufs=1, space="PSUM") as bpool,
        ):
            # ---------------- load inputs ----------------
            a_in = []
            A_view = A_k[:].rearrange("(c p) s -> c p s", p=128)
            for k in range(C):
                t_ = work.tile([128, S], F32, tag=f"a_in{k}")
                nc.sync.dma_start(out=t_[:], in_=A_view[k])
                a_in.append(t_)

            bkT = singles.tile([E, S], F32, tag="bkT")
            nc.sync.dma_start(out=bkT[:], in_=B_k[:].rearrange("s e -> e s"))

            i_row = singles.tile([1, S], F32, tag="i_row")
            nc.sync.dma_start(out=i_row[:], in_=I_k[:].rearrange("(a s) -> a s", a=1))

            oh_sb = singles.tile([E, T * BS], BF16, tag="oh")
            nc.sync.dma_start(out=oh_sb[:], in_=OH[:])

            # ---------------- A = softmax rows -> bf16 chunks ----------------
            a_sb = []
            for k in range(C):
                negmax = small.tile([128, 1], F32, tag="negmax")
                nc.vector.tensor_reduce(negmax[:], a_in[k][:], axis=AX, op=MAX,
                                        negate=True)
                expd = work.tile([128, S], F32, tag="expd")
                nc.scalar.activation(expd[:], a_in[k][:], Exp, bias=negmax[:, 0:1])
                ssum = small.tile([128, 1], F32, tag="ssum")
                nc.vector.tensor_reduce(ssum[:], expd[:], axis=AX, op=ADD)
                sinv = small.tile([128, 1], F32, tag="sinv")
                nc.vector.reciprocal(sinv[:], ssum[:])
                ab = singles.tile([128, S], BF16, tag=f"a_sb{k}")
                nc.vector.tensor_scalar_mul(ab[:], expd[:], sinv[:, 0:1])
                a_sb.append(ab)

            # ---------------- BmT6 = 6 * softmax(B_kernel) transposed --------
            expT = singles.tile([E, S], F32, tag="expT")
            nc.scalar.activation(expT[:], bkT[:], Exp)
            ones6 = singles.tile([E, 1], F32, tag="ones6")
            nc.vector.memset(ones6[:], 1.0)
            denT = ppre.tile([1, S], F32, tag="ps")
            nc.tensor.matmul(denT[:], ones6[:], expT[:], start=True, stop=True)
            denrT = singles.tile([1, S], F32, tag="denrT")
            nc.vector.reciprocal(denrT[:], denT[:])
            nc.vector.tensor_scalar_mul(denrT[:], denrT[:], 6.0)
            denr6 = singles.tile([E, S], F32, tag="denr6")
            nc.gpsimd.partition_broadcast(denr6[:], denrT[:], channels=E)
            bmT6 = singles.tile([E, S], BF16, tag="bmT6")
            nc.vector.tensor_mul(bmT6[:], expT[:], denr6[:])

            # ---------------- I = softmax(I_kernel); BmI = BmT6 * I ----------
            iexp = singles.tile([1, S], F32, tag="iexp")
            nc.scalar.activation(iexp[:], i_row[:], Exp)
            isum = small.tile([1, 1], F32, tag="isum")
            nc.vector.tensor_reduce(isum[:], iexp[:], axis=AX, op=ADD)
            iinv = small.tile([1, 1], F32, tag="iinv")
            nc.vector.reciprocal(iinv[:], isum[:])
            inorm = singles.tile([1, S], F32, tag="inorm")
            nc.vector.tensor_scalar_mul(inorm[:], iexp[:], iinv[:, 0:1])
            i6 = singles.tile([E, S], F32, tag="i6")
            nc.gpsimd.partition_broadcast(i6[:], inorm[:], channels=E)
            denr6i = singles.tile([E, S], F32, tag="denr6i")
            nc.vector.tensor_mul(denr6i[:], denr6[:], i6[:])
            bmI = singles.tile([E, S], BF16, tag="bmI")
            nc.vector.tensor_mul(bmI[:], expT[:], denr6i[:])

            # ---------------- emission table (bf16, SBUF-resident) ----------
            em_sb = singles.tile([128, C, T * BS], BF16, tag="em")
            n_tb = (T * BS) // 512
            for m in range(C):
                lhs = bmT6[:, m * 128:(m + 1) * 128]
                for tb in range(n_tb):
                    ps = ppre.tile([128, 512], F32, tag="ps")
                    nc.tensor.matmul(ps[:], lhs, oh_sb[:, tb * 512:(tb + 1) * 512],
                                     start=True, stop=True)
                    dst = em_sb[:, m, tb * 512:(tb + 1) * 512]
                    if tb % 2 == 0:
                        nc.vector.tensor_copy(dst, ps[:])
                    else:
                        nc.scalar.copy(dst, ps[:])

            # ---------------- constants / state ----------------
            ones_col = singles.tile([128, 1], BF16, tag="ones_col")
            nc.vector.memset(ones_col[:], 1.0)
            ones_row = singles.tile([1, 128], F32, tag="ones_row")
            nc.vector.memset(ones_row[:], 1.0)
            loglik = singles.tile([1, BS], F32, tag="loglik")
            nc.vector.memset(loglik[:], 0.0)
            em2 = singles.tile([128, C, BS], BF16, tag="em2")

            # ---------------- y0 = I * em_0 (via BmI one-hot matmul) --------
            y_cur = ypool.tile([128, C, BS], BF16, tag="y")
            for m in range(C):
                ps0 = gpsum.tile([128, BS], F32, tag="g", name="g")
                nc.tensor.matmul(ps0[:], bmI[:, m * 128:(m + 1) * 128],
                                 oh_sb[:, 0:BS], start=True, stop=True)
                nc.vector.tensor_copy(y_cur[:, m, :], ps0[:])

            # one scan step: y_next[m] = (sum_k A^T[m,k] @ y[k]) * em_src(m)
            def step(y, em_src_fn):
                y_next = ypool.tile([128, C, BS], BF16, tag="y")
                for m in range(C):
                    g = gpsum.tile([128, BS], F32, tag="g", name="g")
                    for k in range(C):
                        nc.tensor.matmul(
                            g[:],
                            a_sb[k][:, m * 128:(m + 1) * 128],
                            y[:, k, :],
                            start=(k == 0), stop=(k == C - 1),
                        )
                    nc.vector.tensor_mul(y_next[:, m, :], g[:], em_src_fn(m))
                return y_next

            # z = colsum(y); loglik += ln z; em2 = em[fold_slice] / z
            def norm_and_fold(y, fold_src_fn):
                zp = zpool.tile([1, BS], F32, tag="z")
                for k in range(C):
                    nc.tensor.matmul(zp[:], ones_col[:], y[:, k, :],
                                     start=(k == 0), stop=(k == C - 1))
                zlog = small.tile([1, BS], F32, tag="zlog")
                nc.scalar.activation(zlog[:], zp[:], Ln)
                nc.vector.tensor_add(loglik[:], loglik[:], zlog[:])
                zrec = small.tile([1, BS], F32, tag="zrec")
                nc.vector.reciprocal(zrec[:], zp[:])
                bp = bpool.tile([128, BS], F32, tag="bp")
                nc.tensor.matmul(bp[:], ones_row[:], zrec[:], start=True,
                                 stop=True)
                for m in range(C):
                    nc.vector.tensor_mul(em2[:, m, :], fold_src_fn(m), bp[:])

            def norm_direct(y):
                zp = zpool.tile([1, BS], F32, tag="z")
                for k in range(C):
                    nc.tensor.matmul(zp[:], ones_col[:], y[:, k, :],
                                     start=(k == 0), stop=(k == C - 1))
                zlog = small.tile([1, BS], F32, tag="zlog")
                nc.scalar.activation(zlog[:], zp[:], Ln)
                nc.vector.tensor_add(loglik[:], loglik[:], zlog[:])
                zrec = small.tile([1, BS], F32, tag="zrec")
                nc.vector.reciprocal(zrec[:], zp[:])
                bp = bpool.tile([128, BS], F32, tag="bp")
                nc.tensor.matmul(bp[:], ones_row[:], zrec[:], start=True,
                                 stop=True)
                ys = yscale.tile([128, C, BS], BF16, tag="ys")
                for m in range(C):
                    nc.vector.tensor_mul(ys[:, m, :], y[:, m, :], bp[:])
                return ys

            # ---------------- peeled head block: t = 1..16 ----------------
            for s in range(1, NP + 1):
                t = s
                y_cur = step(y_cur, lambda m, t=t: em_sb[:, m, t * BS:(t + 1) * BS])
                if s == NP - 1:
                    if fold:
                        norm_and_fold(
                            y_cur,
                            lambda m, t=t: em_sb[:, m, (t + 2) * BS:(t + 3) * BS])
                    else:
                        y_cur = norm_direct(y_cur)

            # ---------------- hardware loop: blocks 1..n_blocks-2 ----------
            # loop var 'base' = block_index * NP * BS (element offset into the
            # time-major free axis); covers t = blk*16+1 .. blk*16+16.
            em_blk = singles.tile([128, C, NP * BS], BF16, tag="em_blk")

            def loop_body(base):
                nonlocal y_cur
                # one dynamic-AP copy of the block's emissions; steps then
                # use static offsets into the staging tile
                nc.vector.tensor_copy(em_blk[:],
                                      em_sb[:, :, ds(base + BS, NP * BS)])
                for s in range(1, NP + 1):
                    if s == 1 and fold:
                        y_cur = step(y_cur, lambda m: em2[:, m, :])
                    else:
                        y_cur = step(
                            y_cur,
                            lambda m, s=s: em_blk[:, m,
                                                  (s - 1) * BS:s * BS])
                    if s == NP - 1:
                        if fold:
                            norm_and_fold(
                                y_cur,
                                lambda m, s=s: em_sb[:, m,
                                                     ds(base + (s + 2) * BS, BS)])
                        else:
                            y_cur = norm_direct(y_cur)

            if hw_loop:
                with tc.For_i(NP * BS, (n_blocks - 1) * NP * BS, NP * BS,
                              hint_engines=(PE_ENG,) if hints else (),
                              staggered_reset=staggered) as base:
                    loop_body(base)
            else:
                for blk in range(1, n_blocks - 1):
                    loop_body(blk * NP * BS)

            # ---------------- peeled tail block: t = T-15..T-1 --------------
            for s in range(1, NP):
                t = (n_blocks - 1) * NP + s
                if s == 1 and fold:
                    y_cur = step(y_cur, lambda m: em2[:, m, :])
                else:
                    y_cur = step(y_cur,
                                 lambda m, t=t: em_sb[:, m, t * BS:(t + 1) * BS])

            # ---------------- finalize ----------------
            zf = zpool.tile([1, BS], F32, tag="z")
            for k in range(C):
                nc.tensor.matmul(zf[:], ones_col[:], y_cur[:, k, :],
                                 start=(k == 0), stop=(k == C - 1))
            zflog = small.tile([1, BS], F32, tag="zlog")
            nc.scalar.activation(zflog[:], zf[:], Ln)
            nc.vector.tensor_add(loglik[:], loglik[:], zflog[:])
            nc.vector.tensor_scalar_add(loglik[:], loglik[:],
                                        -float(T) * math.log(6.0))

            # serialization token: loglik += 0 * chain
            ch_sb = singles.tile([1, BS], F32, tag="ch")
            nc.sync.dma_start(out=ch_sb[:], in_=CH[:])
            chz = small.tile([1, BS], F32, tag="chz")
            nc.vector.tensor_scalar_mul(chz[:], ch_sb[:], 0.0)
            nc.vector.tensor_add(loglik[:], loglik[:], chz[:])

            nc.sync.dma_start(out=OUT[:], in_=loglik[:])

    nc.compile()
    return nc


def make_onehot(obs_shard: np.ndarray, T: int) -> np.ndarray:
    """obs_shard [BS, T] ints -> one-hot [E, T*BS] bf16 with oh[e, t*BS+b]."""
    oh = np.zeros((E, T * BS), dtype=ml_dtypes.bfloat16)
    obs = np.asarray(obs_shard).astype(np.int64)
    tb = np.arange(T)[None, :] * BS + np.arange(BS)[:, None]  # [BS, T]
    oh[obs.reshape(-1), tb.reshape(-1)] = 1.0
    return oh


_CACHED = {}


def _get_program(T):
    if T not in _CACHED:
        t0 = time.time()
        _CACHED[T] = build_program(T, fold=False)
        print(f"[kernel] built bass program T={T} in {time.time()-t0:.1f}s",
              flush=True)
    return _CACHED[T]


def kernel(obs, A_kernel, B_kernel, I_kernel, _trace=False):
    obs = np.asarray(obs)
    A_kernel = np.asarray(A_kernel, dtype=np.float32)
    B_kernel = np.asarray(B_kernel, dtype=np.float32)
    I_kernel = np.asarray(I_kernel, dtype=np.float32)
    Bfull, T = obs.shape
    assert Bfull == B

    nc = _get_program(T)

    in_maps = []
    for c in range(NCORES):
        shard = obs[c * BS:(c + 1) * BS]
        in_maps.append({
            "A_kernel": A_kernel,
            "B_kernel": B_kernel,
            "I_kernel": I_kernel,
            "oh": make_onehot(shard, T),
            "chain": np.zeros((1, BS), np.float32),
        })

    kw = {"trace": True} if _trace else {}
    res = bass_utils.run_bass_kernel_spmd(
        nc, in_maps, core_ids=list(range(NCORES)), **kw,
    )
    out = np.concatenate([r["out"].reshape(BS) for r in res.results])
    kernel._last_result = res
    return out.astype(np.float32)
